# revision 5
# baseline (speedup 1.0000x reference)
"""GQA kernel for 8 Trainium2 NeuronCores (tensor-parallel over heads).

Sharding (per core c of 8):
  - Wq columns 512c..512c+512 (query heads 4c..4c+3), Wk/Wv columns 128c..+128
    (kv head c), Wo rows 512c..+512.
  - Each core computes Q^T/K^T/V for its heads, both softmax orientations
    (natural for the p_attn output + row sums via accum_out; transposed for
    the attention*V matmul), x^T, and a partial output projection.
  - Host sums the 8 partial outputs (+bo) and concatenates p_attn.

All matmuls run as float32r (full-rate fp32 on the PE array).
"""

import sys

sys.path.insert(0, "/opt/trn_rl_repo")

import time
from contextlib import ExitStack

import numpy as np

F32_NP = np.float32

P = 128          # SBUF partitions
S = 2048         # sequence length
D = 4096         # d_model
DK = 128         # head dim
HL = 4           # local (per-core) query heads
KD = D // P      # 32 contraction tiles
ST = S // P      # 16 seq tiles
NC = 8           # cores
SCALE = float(1.0 / np.sqrt(DK))

_STATE: dict = {}
LAST_EXEC_S = None  # wall seconds of the last device execution (set by kernel())


def _build():
    import concourse.mybir as mybir
    import concourse.tile as tile
    import concourse.bacc as bacc
    from concourse.masks import make_identity

    F32 = mybir.dt.float32
    F32R = mybir.dt.float32r
    ADD = mybir.AluOpType.add
    MULT = mybir.AluOpType.mult
    EXP = mybir.ActivationFunctionType.Exp
    COPY = mybir.ActivationFunctionType.Copy

    nc = bacc.Bacc("TRN2", target_bir_lowering=False)

    q_in = nc.declare_dram_parameter("query", [S, D], F32, isOutput=False)
    k_in = nc.declare_dram_parameter("key", [S, D], F32, isOutput=False)
    v_in = nc.declare_dram_parameter("value", [S, D], F32, isOutput=False)
    wq_in = nc.declare_dram_parameter("wq", [D, HL * DK], F32, isOutput=False)
    wk_in = nc.declare_dram_parameter("wk", [D, DK], F32, isOutput=False)
    wv_in = nc.declare_dram_parameter("wv", [D, DK], F32, isOutput=False)
    wo_in = nc.declare_dram_parameter("wo", [HL * DK, D], F32, isOutput=False)
    bq_in = nc.declare_dram_parameter("bq", [HL * DK, 1], F32, isOutput=False)
    bk_in = nc.declare_dram_parameter("bk", [DK, 1], F32, isOutput=False)
    bv_in = nc.declare_dram_parameter("bv", [DK, 1], F32, isOutput=False)
    p_out = nc.declare_dram_parameter("p_attn", [HL, S, S], F32, isOutput=True)
    o_out = nc.declare_dram_parameter("out_partial", [S, D], F32, isOutput=True)

    with tile.TileContext(nc) as tc, ExitStack() as ctx:
        const = ctx.enter_context(tc.tile_pool(name="const", bufs=1))
        ident_f = const.tile([P, P], F32)
        make_identity(nc, ident_f)
        ident = const.tile([P, P], F32R)
        nc.vector.tensor_copy(ident[:], ident_f[:])
        ones_f = const.tile([1, P], F32)
        nc.vector.memset(ones_f[:], 1.0)
        ones_r = const.tile([1, P], F32R)
        nc.vector.tensor_copy(ones_r[:], ones_f[:])

        bq_sb = const.tile([P, HL], F32)
        for g in range(HL):
            nc.sync.dma_start(out=bq_sb[:, g : g + 1], in_=bq_in[g * P : (g + 1) * P, :])
        bk_sb = const.tile([P, 1], F32)
        nc.sync.dma_start(out=bk_sb[:], in_=bk_in[:])
        bv_sb = const.tile([P, 1], F32)
        nc.sync.dma_start(out=bv_sb[:], in_=bv_in[:])

        persist = ctx.enter_context(tc.tile_pool(name="persist", bufs=1))
        qT = persist.tile([P, HL, S], F32R)   # Q^T per head  (4 MB)
        kT = persist.tile([P, S], F32R)       # K^T           (1 MB)
        vN = persist.tile([P, ST, DK], F32R)  # V natural     (1 MB)

        # ---------------- phase 1: projections -----------------------------
        def project(src_ap, w_dram, n_out, bias_col, out_write):
            """out_write(g, sc, psum_tile) receives [dout_tile g, seq 256]."""
            with ExitStack() as c2:
                wpool = c2.enter_context(tc.tile_pool(name="wpool", bufs=1))
                w_sb = wpool.tile([P, KD, n_out * P], F32R, tag="w")
                for kt in range(KD):
                    nc.sync.dma_start(
                        out=w_sb[:, kt, :],
                        in_=w_dram[kt * P : (kt + 1) * P, :].bitcast(F32R),
                    )
                natp = c2.enter_context(tc.tile_pool(name="natp", bufs=2))
                actp = c2.enter_context(tc.tile_pool(name="actp", bufs=1))
                tpp = c2.enter_context(tc.tile_pool(name="tpp", bufs=2, space="PSUM"))
                pjp = c2.enter_context(
                    tc.tile_pool(name="pjp", bufs=max(2, n_out), space="PSUM")
                )
                for sc in range(S // 256):
                    actT = actp.tile([P, KD, 256], F32R, tag="actT")
                    for sub in range(2):
                        st = sc * 2 + sub
                        nat = natp.tile([P, D], F32, tag="nat")
                        nc.sync.dma_start(
                            out=nat[:],
                            in_=src_ap[st * P : (st + 1) * P, :],
                        )
                        for kt in range(KD):
                            tp = tpp.tile([P, P], F32, tag="tp")
                            nc.tensor.transpose(
                                tp[:], nat[:, kt * P : (kt + 1) * P], ident_f[:]
                            )
                            nc.vector.tensor_copy(
                                actT[:, kt, sub * P : (sub + 1) * P], tp[:]
                            )
                    for g in range(n_out):
                        ps = pjp.tile([P, 256], F32, tag="pj")
                        for kt in range(KD):
                            nc.tensor.matmul(
                                ps[:],
                                w_sb[:, kt, g * P : (g + 1) * P],
                                actT[:, kt, :],
                                start=(kt == 0),
                                stop=(kt == KD - 1),
                            )
                        out_write(g, sc, ps)

        def q_write(g, sc, ps):
            nc.vector.tensor_scalar(
                out=qT[:, g, sc * 256 : (sc + 1) * 256],
                in0=ps[:],
                scalar1=bq_sb[:, g : g + 1],
                scalar2=None,
                op0=ADD,
            )

        project(q_in, wq_in, HL, bq_sb, q_write)

        def k_write(g, sc, ps):
            nc.vector.tensor_scalar(
                out=kT[:, sc * 256 : (sc + 1) * 256],
                in0=ps[:],
                scalar1=bk_sb[:, 0:1],
                scalar2=None,
                op0=ADD,
            )

        project(k_in, wk_in, 1, bk_sb, k_write)

        with ExitStack() as c2:
            vtp = c2.enter_context(tc.tile_pool(name="vtp", bufs=1))
            vT = vtp.tile([P, S], F32)

            def v_write(g, sc, ps):
                nc.vector.tensor_scalar(
                    out=vT[:, sc * 256 : (sc + 1) * 256],
                    in0=ps[:],
                    scalar1=bv_sb[:, 0:1],
                    scalar2=None,
                    op0=ADD,
                )

            project(v_in, wv_in, 1, bv_sb, v_write)

            # V^T -> V natural
            vtpp = c2.enter_context(tc.tile_pool(name="vtpp", bufs=2, space="PSUM"))
            for st in range(ST):
                tp = vtpp.tile([P, P], F32, tag="vtp")
                nc.tensor.transpose(tp[:], vT[:, st * P : (st + 1) * P], ident_f[:])
                nc.vector.tensor_copy(vN[:, st, :], tp[:])

        # ---------------- phase 2: attention --------------------------------
        xtpool = ctx.enter_context(tc.tile_pool(name="xtpool", bufs=1))
        xT = xtpool.tile([P, HL, S], F32R)   # normalized x^T (4 MB)
        with ExitStack() as c2:
            sTp = c2.enter_context(tc.tile_pool(name="sTp", bufs=2, space="PSUM"))
            xpp = c2.enter_context(tc.tile_pool(name="xpp", bufs=2, space="PSUM"))
            sNp = c2.enter_context(tc.tile_pool(name="sNp", bufs=2, space="PSUM"))
            rtp = c2.enter_context(tc.tile_pool(name="rtp", bufs=1, space="PSUM"))
            bcp = c2.enter_context(tc.tile_pool(name="bcp", bufs=1, space="PSUM"))
            pTp = c2.enter_context(tc.tile_pool(name="pTp", bufs=1))
            pTfp = c2.enter_context(tc.tile_pool(name="pTfp", bufs=3))
            pNp = c2.enter_context(tc.tile_pool(name="pNp", bufs=2))
            xup = c2.enter_context(tc.tile_pool(name="xup", bufs=1))
            smp = c2.enter_context(tc.tile_pool(name="smp", bufs=4))
            rcp = c2.enter_context(tc.tile_pool(name="rcp", bufs=1))

            xTu = xup.tile([P, HL, S], F32)

            for h in range(HL):
                recTh = rcp.tile([1, S], F32, tag="recT")
                for qc in range(S // 512):
                    # transposed scores -> exp -> pT (for attn@V)
                    pT = pTp.tile([P, ST, 512], F32R, tag="pT")
                    for kt in range(ST):
                        sT = sTp.tile([P, 512], F32, tag="sT")
                        nc.tensor.matmul(
                            sT[:],
                            kT[:, kt * P : (kt + 1) * P],
                            qT[:, h, qc * 512 : (qc + 1) * 512],
                            start=True,
                            stop=True,
                        )
                        pTf = pTfp.tile([P, 512], F32, tag="pTf")
                        nc.scalar.activation(pTf[:], sT[:], EXP, scale=SCALE)
                        nc.vector.tensor_copy(pT[:, kt, :], pTf[:])
                    # x^T accumulation (unnormalized)
                    xps = xpp.tile([P, 512], F32, tag="xps")
                    for kt in range(ST):
                        nc.tensor.matmul(
                            xps[:],
                            vN[:, kt, :],
                            pT[:, kt, :],
                            start=(kt == 0),
                            stop=(kt == ST - 1),
                        )
                    nc.scalar.activation(
                        xTu[:, h, qc * 512 : (qc + 1) * 512], xps[:], COPY
                    )
                    # natural scores -> exp+accum -> normalize -> p_attn out
                    for qt in range(4):
                        qtg = qc * 4 + qt
                        pN = pNp.tile([P, S], F32, tag="pN")
                        sums4 = smp.tile([P, 4], F32, tag="sums")
                        for kc in range(4):
                            sN = sNp.tile([P, 512], F32, tag="sN")
                            nc.tensor.matmul(
                                sN[:],
                                qT[:, h, qtg * P : (qtg + 1) * P],
                                kT[:, kc * 512 : (kc + 1) * 512],
                                start=True,
                                stop=True,
                            )
                            nc.scalar.activation(
                                pN[:, kc * 512 : (kc + 1) * 512],
                                sN[:],
                                EXP,
                                scale=SCALE,
                                accum_out=sums4[:, kc : kc + 1],
                            )
                        sumt = smp.tile([P, 1], F32, tag="sumt")
                        nc.vector.tensor_reduce(
                            sumt[:], sums4[:], mybir.AxisListType.X, ADD
                        )
                        rec = smp.tile([P, 1], F32, tag="rec")
                        nc.vector.reciprocal(rec[:], sumt[:])
                        nc.vector.tensor_scalar(
                            out=pN[:],
                            in0=pN[:],
                            scalar1=rec[:],
                            scalar2=None,
                            op0=MULT,
                        )
                        nc.sync.dma_start(
                            out=p_out[h, qtg * P : (qtg + 1) * P, :], in_=pN[:]
                        )
                        # transposed reciprocal row for x^T normalization
                        rt = rtp.tile([1, P], F32, tag="rt")
                        nc.tensor.matmul(
                            rt[:], rec[:], ident_f[:], start=True, stop=True
                        )
                        nc.vector.tensor_copy(
                            recTh[0:1, qtg * P : (qtg + 1) * P], rt[:]
                        )
                    # normalize x^T for this q-chunk
                    bc = bcp.tile([P, 512], F32, tag="bc")
                    nc.tensor.matmul(
                        bc[:],
                        ones_f[:, :],
                        recTh[0:1, qc * 512 : (qc + 1) * 512],
                        start=True,
                        stop=True,
                    )
                    nc.vector.tensor_tensor(
                        out=xT[:, h, qc * 512 : (qc + 1) * 512],
                        in0=xTu[:, h, qc * 512 : (qc + 1) * 512],
                        in1=bc[:],
                        op=MULT,
                    )

        # ---------------- phase 3: output projection ------------------------
        with ExitStack() as c2:
            wop = c2.enter_context(tc.tile_pool(name="wop", bufs=1))
            wo_sb = wop.tile([P, HL, D], F32R)
            for kt in range(HL):
                nc.sync.dma_start(
                    out=wo_sb[:, kt, :],
                    in_=wo_in[kt * P : (kt + 1) * P, :].bitcast(F32R),
                )
            opp = c2.enter_context(tc.tile_pool(name="opp", bufs=4, space="PSUM"))
            obp = c2.enter_context(tc.tile_pool(name="obp", bufs=4))
            for st in range(ST):
                for dc in range(D // 512):
                    ops = opp.tile([P, 512], F32, tag="ops")
                    for kt in range(HL):
                        nc.tensor.matmul(
                            ops[:],
                            xT[:, kt, st * P : (st + 1) * P],
                            wo_sb[:, kt, dc * 512 : (dc + 1) * 512],
                            start=(kt == 0),
                            stop=(kt == HL - 1),
                        )
                    ob = obp.tile([P, 512], F32, tag="ob")
                    nc.vector.tensor_copy(ob[:], ops[:])
                    nc.sync.dma_start(
                        out=o_out[st * P : (st + 1) * P, dc * 512 : (dc + 1) * 512],
                        in_=ob[:],
                    )

    nc.finalize()
    return nc


def _get_runner():
    """Build (once) a cached jitted SPMD runner over 8 cores."""
    if "runner" in _STATE:
        return _STATE["runner"]

    import jax
    import numpy as np_
    from jax.sharding import Mesh, PartitionSpec
    from jax.experimental.shard_map import shard_map
    import concourse.mybir as mybir
    import concourse.bass2jax as bass2jax
    from concourse.bass2jax import _bass_exec_p, partition_id_tensor

    nc = _build()
    bass2jax.install_neuronx_cc_hook()

    partition_name = nc.partition_id_tensor.name if nc.partition_id_tensor else None
    in_names, out_names, out_avals, zero_shapes = [], [], [], []
    for alloc in nc.m.functions[0].allocations:
        if not isinstance(alloc, mybir.MemoryLocationSet):
            continue
        name = alloc.memorylocations[0].name
        if alloc.kind == "ExternalInput":
            if name != partition_name:
                in_names.append(name)
        elif alloc.kind == "ExternalOutput":
            out_names.append(name)
            shape = tuple(alloc.tensor_shape)
            dtype = mybir.dt.np(alloc.dtype)
            out_avals.append(jax.core.ShapedArray(shape, dtype))
            zero_shapes.append((shape, dtype))
    n_params = len(in_names)
    n_outs = len(out_avals)
    all_in_names = list(in_names) + list(out_names)
    if partition_name is not None:
        all_in_names.append(partition_name)

    def _body(*args):
        operands = list(args)
        if partition_name is not None:
            operands.append(partition_id_tensor())
        outs = _bass_exec_p.bind(
            *operands,
            out_avals=tuple(out_avals),
            in_names=tuple(all_in_names),
            out_names=tuple(out_names),
            lowering_input_output_aliases=(),
            sim_require_finite=True,
            sim_require_nnan=True,
            nc=nc,
        )
        return tuple(outs)

    devices = jax.devices()[:NC]
    mesh = Mesh(np_.asarray(devices), ("core",))
    in_specs = (PartitionSpec("core"),) * (n_params + n_outs)
    out_specs = (PartitionSpec("core"),) * n_outs
    sharded = jax.jit(
        shard_map(_body, mesh=mesh, in_specs=in_specs, out_specs=out_specs,
                  check_rep=False),
        keep_unused=True,
    )
    _STATE["runner"] = (sharded, in_names, out_names, zero_shapes)
    return _STATE["runner"]


def _shard_inputs(inputs):
    """Build the concatenated (8*dim0, ...) arrays for each DRAM parameter."""
    q = np.ascontiguousarray(inputs["query"].reshape(S, D), F32_NP)
    k = np.ascontiguousarray(inputs["key"].reshape(S, D), F32_NP)
    v = np.ascontiguousarray(inputs["value"].reshape(S, D), F32_NP)
    Wq = np.asarray(inputs["Wq"], F32_NP)
    Wk = np.asarray(inputs["Wk"], F32_NP)
    Wv = np.asarray(inputs["Wv"], F32_NP)
    Wo = np.asarray(inputs["Wo"], F32_NP)
    bq = np.asarray(inputs["bq"], F32_NP)
    bk = np.asarray(inputs["bk"], F32_NP)
    bv = np.asarray(inputs["bv"], F32_NP)

    per_core = {name: [] for name in
                ["query", "key", "value", "wq", "wk", "wv", "wo", "bq", "bk", "bv"]}
    for c in range(NC):
        qs, ks = c * HL * DK, (c + 1) * HL * DK
        kvs, kve = c * DK, (c + 1) * DK
        per_core["query"].append(q)
        per_core["key"].append(k)
        per_core["value"].append(v)
        per_core["wq"].append(np.ascontiguousarray(Wq[:, qs:ks]))
        per_core["wk"].append(np.ascontiguousarray(Wk[:, kvs:kve]))
        per_core["wv"].append(np.ascontiguousarray(Wv[:, kvs:kve]))
        per_core["wo"].append(np.ascontiguousarray(Wo[qs:ks, :]))
        per_core["bq"].append(np.ascontiguousarray(bq[qs:ks].reshape(-1, 1)))
        per_core["bk"].append(np.ascontiguousarray(bk[kvs:kve].reshape(-1, 1)))
        per_core["bv"].append(np.ascontiguousarray(bv[kvs:kve].reshape(-1, 1)))
    return {name: np.concatenate(arrs, axis=0) for name, arrs in per_core.items()}


def kernel(**inputs):
    global LAST_EXEC_S
    import jax

    sharded, in_names, out_names, zero_shapes = _get_runner()
    shards = _shard_inputs(inputs)
    concat_in = [shards[name] for name in in_names]
    concat_zeros = [
        np.zeros((NC * shape[0], *shape[1:]), dtype) for shape, dtype in zero_shapes
    ]
    args = [jax.device_put(a) for a in concat_in + concat_zeros]
    for a in args:
        a.block_until_ready()
    t0 = time.perf_counter()
    out_arrs = sharded(*args)
    for o in out_arrs:
        o.block_until_ready()
    LAST_EXEC_S = time.perf_counter() - t0

    outs = {name: np.asarray(out_arrs[i]) for i, name in enumerate(out_names)}
    # p_attn: (8*HL, S, S) -> (1, 32, S, S)
    p_attn = outs["p_attn"].reshape(NC * HL, S, S)[None]
    # output: sum of partials + bo
    partials = outs["out_partial"].reshape(NC, S, D)
    output = partials.sum(axis=0, dtype=np.float64).astype(F32_NP)
    output = output + np.asarray(inputs["bo"], F32_NP)[None, :]
    return output[None], p_attn
